# revision 1
# baseline (speedup 1.0000x reference)
"""Complex-valued causal attention head on 8 Trainium2 NeuronCores.

Math (per batch element, fp32 reference):
    q/k/v = complex_linear(x, W*)          # contract C=1024 -> H=64
    wr + i*wi = q @ conj(k)^T              # contract H
    mag = sqrt(wr^2 + wi^2 + 1e-4) / sqrt(H)
    wei = softmax(causal_mask(mag))
    out = wei @ v   (real and imag parts separately)

Sharding: data-parallel over batch B=8 -> one batch element per core, weights
replicated, no collectives.

v3 (bf16): all matmul operands bf16 (PSUM accumulation stays fp32), x shipped
bf16 (halves HBM traffic), Q2 = [-qi; qr] produced by a 128x128 permutation
matmul Mt^T @ Q+ instead of a 4th projection (saves 30k PE cycles), score
PSUM tiles are [128, 1024] pair-tiles spanning 2 banks so Square/copy run one
call per 2 blocks, ln/exp/exp coalesced over 4-block groups, PSUM laid out as
one 6-bank rotation + 2 accumulator banks.

Per-core dataflow (T=2048, C=1024, H=64):
  - Projections: pre-stacked weight pairs [Wr|Wi] / [-Wi|Wr] with PSUM
    accumulation; outputs H-stacked transposed: K+=[kr;ki], Q+=[qr;qi],
    V+=[vr;vi], each [128, T-chunk]; Q2 via permutation matmul.
  - Scores TRANSPOSED [tk, tq]: psRe = K+^T Q+, psIm = K+^T Q2 (sign of im
    dies in squaring); mag^2 via ACT Square (re, from PSUM) + DVE copy/mult
    (im, bf16 2x) + gpsimd add; then per 4-block group on ACT:
        p = exp(exp(0.5*ln(s+eps) + ln(H^-0.5)))
    (square/ln/exp share one ACT table set -> no table reloads; ln/exp
    intermediates kept fp32, p emitted bf16).
  - Causal mask on diagonal blocks via gpsimd affine_select (p:=0).
  - Row sums via ones-matmul on PE; PV accumulates out^T [h2, tq].
  - out^T PE-transposed back to natural [t, h2]; 1/rowsum rides the
    PSUM->SBUF copy via tensor_scalar_mul; DMA out fp32.
"""

import numpy as np

B, T, C, H = 8, 2048, 1024, 64
H2 = 2 * H            # stacked real|imag head dim = 128
P = 128               # partitions
NCHUNK = 4            # T / 512
CH = T // NCHUNK      # 512 tq columns per chunk
TB = T // P           # 16 tk blocks
EPS = 1e-4
C_SCALE = float(H) ** -0.5
PAIR = 2 * CH         # pair-tile width (2 tk blocks = 1024)
GRP = 4               # tk blocks per ln/exp/exp group

_BUILT = None


def _build(loop_n=None):
    import contextlib

    import concourse.bass as bass
    import concourse.mybir as mybir
    import concourse.tile as tile

    f32 = mybir.dt.float32
    bf16 = mybir.dt.bfloat16
    AF = mybir.ActivationFunctionType

    nc = bass.Bass(trn_type="TRN2")

    # x pre-transposed AND partition-major bf16: [chunk, p, cc, t]
    xr_d = nc.dram_tensor("xT_real", [NCHUNK, P, C // P, CH], bf16, kind="ExternalInput").ap()
    xi_d = nc.dram_tensor("xT_imag", [NCHUNK, P, C // P, CH], bf16, kind="ExternalInput").ap()
    # 6 host-stacked weight blocks, each (C, H2):
    # 0:S1q=[Wqr|Wqi] 1:S2q=[-Wqi|Wqr] 2:S1k 3:S2k 4:S1v 5:S2v
    wst_d = nc.dram_tensor("wstacks", [P, 6, C // P, H2], bf16, kind="ExternalInput").ap()
    # consts bf16: [:, :128]=eye, [:, 128:256]=Mt (lhsT for Q2=M@Q+), [:, 256]=ones
    consts_d = nc.dram_tensor("consts", [P, 2 * P + 1], bf16, kind="ExternalInput").ap()

    # packed output [chunk, p, tb, h2] fp32; host unpacks to (T, H) r/i halves
    out_d = nc.dram_tensor("out_pk", [NCHUNK, P, 4, H2], f32, kind="ExternalOutput").ap()

    CC = C // P  # 8 contraction chunks

    with tile.TileContext(nc) as tc:
        ctx = contextlib.ExitStack()
        with ctx:
            if loop_n is not None:
                ctx.enter_context(tc.For_i(0, loop_n, 1))
            singles = ctx.enter_context(tc.tile_pool(name="singles", bufs=1))
            xt_p = ctx.enter_context(tc.tile_pool(name="xt", bufs=2))
            qc_p = ctx.enter_context(tc.tile_pool(name="qc", bufs=2))
            elw_p = ctx.enter_context(tc.tile_pool(name="elw", bufs=1))
            im_p = ctx.enter_context(tc.tile_pool(name="imp", bufs=3))
            mg_p = ctx.enter_context(tc.tile_pool(name="mg", bufs=2))
            fin_p = ctx.enter_context(tc.tile_pool(name="fin", bufs=2))

            # PSUM budget (8 banks): scps pair-tiles 3 bufs x 2 banks = 6,
            # accps (out+sums) = 2; chunk-boundary tiles reuse scps slots
            scps = ctx.enter_context(tc.tile_pool(name="scps", bufs=2, space="PSUM"))
            accps = ctx.enter_context(tc.tile_pool(name="accps", bufs=1, space="PSUM"))
            finps = ctx.enter_context(tc.tile_pool(name="finps", bufs=1, space="PSUM"))

            # ---- constants ----
            consts_t = singles.tile([P, 2 * P + 1], bf16)
            nc.sync.dma_start(consts_t, consts_d)
            eye_b = consts_t[:, 0:P]
            mt_b = consts_t[:, P:2 * P]
            ones_col = consts_t[:, 2 * P:2 * P + 1]
            one1_b = consts_t[0:1, 2 * P:2 * P + 1]

            bias_eps = singles.tile([P, 1], f32)
            nc.vector.memset(bias_eps, EPS)
            bias_lnc = singles.tile([P, 1], f32)
            nc.vector.memset(bias_lnc, float(np.log(C_SCALE)))
            bias_zero = singles.tile([P, 1], f32)
            nc.vector.memset(bias_zero, 0.0)

            wst = singles.tile([P, 6, CC, H2], bf16)
            nc.sync.dma_start(wst, wst_d)
            S1q, S2q, S1k, S2k, S1v, S2v = (wst[:, i] for i in range(6))

            # ---- persistent per-batch buffers ----
            k_all = singles.tile([P, T], bf16)       # K+ = [kr^T; ki^T]
            v_nat = singles.tile([P, TB, H2], bf16)  # V natural [t, h2] blocks

            for j in range(NCHUNK):
                # ---------- load xT chunk (pre-transposed on host) ----------
                xt_r = xt_p.tile([P, CC, CH], bf16, tag="xtr")
                xt_i = xt_p.tile([P, CC, CH], bf16, tag="xti")
                c0, c1 = j * CH, (j + 1) * CH
                for h in range(2):
                    cs = slice(4 * h, 4 * h + 4)
                    nc.sync.dma_start(xt_r[:, cs], xr_d[j][:, cs])
                    nc.scalar.dma_start(xt_i[:, cs], xi_d[j][:, cs])

                # ---------- projections (PSUM-accumulated complex) ----------
                def proj(stack_r, stack_i):
                    ps = scps.tile([P, PAIR], f32, tag="scps")
                    for cc in range(CC):
                        nc.tensor.matmul(ps[:, 0:CH], stack_r[:, cc], xt_r[:, cc],
                                         start=(cc == 0), stop=False)
                    for cc in range(CC):
                        nc.tensor.matmul(ps[:, 0:CH], stack_i[:, cc], xt_i[:, cc],
                                         start=False, stop=(cc == CC - 1))
                    return ps

                ps_k = proj(S1k, S2k)
                nc.vector.tensor_copy(k_all[:, c0:c1], ps_k[:, 0:CH])

                ps_q = proj(S1q, S2q)
                q_c = qc_p.tile([P, CH], bf16, tag="qc")
                nc.vector.tensor_copy(q_c, ps_q[:, 0:CH])

                # Q2 = M @ Q+ via permutation matmul (lhsT = Mt = M^T)
                ps_q2 = scps.tile([P, PAIR], f32, tag="scps")
                nc.tensor.matmul(ps_q2[:, 0:CH], mt_b, q_c, start=True, stop=True)
                q2_c = qc_p.tile([P, CH], bf16, tag="q2c")
                nc.vector.tensor_copy(q2_c, ps_q2[:, 0:CH])

                ps_v = proj(S1v, S2v)
                vt_c = qc_p.tile([P, CH], bf16, tag="vtc")
                nc.vector.tensor_copy(vt_c, ps_v[:, 0:CH])
                # V+ [h2, t] -> natural [t, h2] blocks
                ps_vn = finps.tile([P, 4, H2], bf16, tag="vno")
                for t4 in range(4):
                    nc.tensor.transpose(
                        ps_vn[:, t4], vt_c[:, t4 * P:(t4 + 1) * P], eye_b)
                nc.vector.tensor_copy(v_nat[:, j * 4:(j + 1) * 4], ps_vn)

                # ---------- scores / softmax / PV over tk blocks ----------
                ps_out = accps.tile([P, CH], f32, tag="outps")
                ps_sums = accps.tile([1, CH], f32, tag="sumps")
                nblk = 4 * (j + 1)
                # flat per-chunk elementwise buffers (constant max size)
                sq1 = elw_p.tile([P, TB * CH], f32, tag="sq1")
                s_t = elw_p.tile([P, TB * CH], f32, tag="st")
                p_t = elw_p.tile([P, TB * CH], bf16, tag="pt")

                for qd in range(nblk // 2):
                    ps_re = scps.tile([P, PAIR], f32, tag="scps")
                    ps_im = scps.tile([P, PAIR], f32, tag="scps")
                    for b2 in range(2):
                        i = qd * 2 + b2
                        kT = k_all[:, i * P:(i + 1) * P]
                        sl = slice(b2 * CH, (b2 + 1) * CH)
                        nc.tensor.matmul(ps_re[:, sl], kT, q_c,
                                         start=True, stop=True)
                        nc.tensor.matmul(ps_im[:, sl], kT, q2_c,
                                         start=True, stop=True)
                    psl = slice(2 * qd * CH, (2 * qd + 2) * CH)
                    # re^2 on ACT (pair-wide, single PSUM operand)
                    nc.scalar.activation(sq1[:, psl], ps_re,
                                         AF.Square, bias=bias_zero, scale=1.0)
                    # im -> SBUF bf16, square on DVE (2x), add on gpsimd
                    im_s = im_p.tile([P, PAIR], f32, tag="ims")
                    nc.vector.tensor_copy(im_s, ps_im)
                    sq2 = im_p.tile([P, PAIR], f32, tag="sq2")
                    nc.vector.tensor_tensor(sq2, im_s, im_s,
                                            mybir.AluOpType.mult)
                    if qd % 2 == 0:
                        nc.gpsimd.tensor_add(s_t[:, psl], sq1[:, psl], sq2)
                    else:
                        nc.vector.tensor_tensor(s_t[:, psl], sq1[:, psl], sq2,
                                                mybir.AluOpType.add)

                    if qd % 2 == 1:
                        # ln/exp/exp over the finished 4-block group
                        g0 = 2 * (qd - 1)
                        gsl = slice(g0 * CH, (g0 + GRP) * CH)
                        m_t = mg_p.tile([P, GRP * CH], f32, tag="mt")
                        nc.scalar.activation(m_t, s_t[:, gsl], AF.Ln,
                                             bias=bias_eps, scale=1.0)
                        nc.scalar.activation(m_t, m_t, AF.Exp,
                                             bias=bias_lnc, scale=0.5)
                        nc.scalar.activation(p_t[:, gsl], m_t, AF.Exp,
                                             bias=bias_zero, scale=1.0)
                        for b in range(g0, g0 + GRP):
                            p_blk = p_t[:, b * CH:(b + 1) * CH]
                            if b >= 4 * j:  # diagonal: zero where tq < tk
                                nc.gpsimd.affine_select(
                                    out=p_blk, in_=p_blk,
                                    compare_op=mybir.AluOpType.is_ge,
                                    fill=0.0,
                                    base=j * CH - b * P,
                                    pattern=[[1, CH]],
                                    channel_multiplier=-1)
                            nc.tensor.matmul(ps_sums, ones_col, p_blk,
                                             start=(b == 0), stop=(b == nblk - 1))
                            nc.tensor.matmul(ps_out, v_nat[:, b], p_blk,
                                             start=(b == 0), stop=(b == nblk - 1))

                # ---------- finalize chunk ----------
                outT = fin_p.tile([P, CH], bf16, tag="outT")
                nc.vector.tensor_copy(outT, ps_out)
                sums_sb = fin_p.tile([1, CH], bf16, tag="sums")
                nc.vector.tensor_copy(sums_sb, ps_sums)

                ps_on = finps.tile([P, 4, H2], bf16, tag="vno")
                for t4 in range(4):
                    nc.tensor.transpose(
                        ps_on[:, t4], outT[:, t4 * P:(t4 + 1) * P], eye_b)
                ps_rs = finps.tile([P, 4], f32, tag="rsps")
                for t4 in range(4):
                    nc.tensor.matmul(ps_rs[:, t4:t4 + 1],
                                     sums_sb[0:1, t4 * P:(t4 + 1) * P],
                                     one1_b, start=True, stop=True)
                recip = fin_p.tile([P, 4], f32, tag="recip")
                nc.vector.reciprocal(recip, ps_rs)

                onat = fin_p.tile([P, 4, H2], f32, tag="onat")
                for t4 in range(4):
                    nc.vector.tensor_scalar_mul(
                        onat[:, t4], ps_on[:, t4], recip[:, t4:t4 + 1])
                nc.sync.dma_start(out_d[j], onat)

    _split_multiwaits(nc)
    return nc


def _split_multiwaits(nc):
    """This toolchain's walrus accepts at most ONE sync-wait per instruction;
    Tile's sem-assignment can attach several. Hoist all-but-one wait onto
    standalone InstEventSemaphore carriers (what bass's wait_ge emits)."""
    import concourse.mybir as mybir

    n_split = 0
    for f in nc.m.functions:
        for bb in f.blocks:
            out = []
            for inst in bb.instructions:
                si = inst.sync_info
                if si is not None and si.on_wait and len(si.on_wait) > 1:
                    waits = list(si.on_wait)
                    for w in waits[:-1]:
                        carrier = mybir.InstEventSemaphore(
                            name=f"{inst.name}_wsplit{n_split}", ins=[], outs=[])
                        carrier.engine = inst.engine
                        carrier.sync_info = mybir.SyncInfo(
                            on_wait=[w], on_update=[])
                        out.append(carrier)
                        n_split += 1
                    inst.sync_info = mybir.SyncInfo(
                        on_wait=[waits[-1]], on_update=list(si.on_update))
                out.append(inst)
            bb.instructions = out
    return n_split


def _host_prep(Wk_r, Wk_i, Wq_r, Wq_i, Wv_r, Wv_i):
    import ml_dtypes

    def s1(wr, wi):
        return np.concatenate([wr, wi], axis=1)

    def s2(wr, wi):
        return np.concatenate([-wi, wr], axis=1)

    wst = np.stack([
        s1(Wq_r, Wq_i), s2(Wq_r, Wq_i),
        s1(Wk_r, Wk_i), s2(Wk_r, Wk_i),
        s1(Wv_r, Wv_i), s2(Wv_r, Wv_i),
    ]).astype(np.float32)
    # partition-major: (6, C, H2) -> (p, s, cc, h2)
    wst = np.ascontiguousarray(
        wst.reshape(6, C // P, P, H2).transpose(2, 0, 1, 3)
    ).astype(ml_dtypes.bfloat16)
    # Mt = M^T where Q2 = M @ Q+, M = [[0, -I],[I, 0]] (rows: -qi then qr)
    Mt = np.zeros((P, P), np.float32)
    Mt[H:, 0:H] = -np.eye(H, dtype=np.float32)   # M[0:H, H:] = -I -> Mt[H:, 0:H]
    Mt[0:H, H:] = np.eye(H, dtype=np.float32)    # M[H:, 0:H] = I  -> Mt[0:H, H:]
    consts = np.concatenate(
        [np.eye(P, dtype=np.float32), Mt, np.ones((P, 1), np.float32)],
        axis=1).astype(ml_dtypes.bfloat16)
    return wst, np.ascontiguousarray(consts)


def prep_in_maps(x_real, x_imag, Wk_r, Wk_i, Wq_r, Wq_i, Wv_r, Wv_i):
    import ml_dtypes

    wst, consts = _host_prep(
        np.asarray(Wk_r), np.asarray(Wk_i), np.asarray(Wq_r),
        np.asarray(Wq_i), np.asarray(Wv_r), np.asarray(Wv_i))
    x_real = np.asarray(x_real, dtype=np.float32)
    x_imag = np.asarray(x_imag, dtype=np.float32)

    def xprep(xb):
        # (T, C) -> xT (C, T) -> [chunk, p, cc, t] partition-major, bf16
        return np.ascontiguousarray(
            xb.T.reshape(C // P, P, NCHUNK, CH).transpose(2, 1, 0, 3)
        ).astype(ml_dtypes.bfloat16)

    return [
        {
            "xT_real": xprep(x_real[b]),
            "xT_imag": xprep(x_imag[b]),
            "wstacks": wst,
            "consts": consts,
        }
        for b in range(B)
    ]


def kernel(x_real, x_imag, Wk_r, Wk_i, Wq_r, Wq_i, Wv_r, Wv_i, _trace=False):
    global _BUILT
    from concourse.bass_utils import run_bass_kernel_spmd

    if _BUILT is None:
        _BUILT = _build()
    nc = _BUILT

    in_maps = prep_in_maps(x_real, x_imag, Wk_r, Wk_i,
                           Wq_r, Wq_i, Wv_r, Wv_i)
    res = run_bass_kernel_spmd(nc, in_maps, core_ids=list(range(B)),
                               trace=_trace)

    def unpack(pk):
        # [chunk, p, tb, h2] -> (T, H2)
        full = pk.transpose(0, 2, 1, 3).reshape(T, H2)
        return full[:, 0:H], full[:, H:H2]

    outs = [unpack(res.results[b]["out_pk"]) for b in range(B)]
    out_r = np.ascontiguousarray(np.stack([o[0] for o in outs]))
    out_i = np.ascontiguousarray(np.stack([o[1] for o in outs]))
    if _trace:
        kernel._last_results = res
    return out_r, out_i

